# revision 13
# baseline (speedup 1.0000x reference)
"""Multi-head attention (RoPE, causal) Trainium2 kernel, SPMD over 8 NeuronCores.

Problem: x[2,2048,1024] @ {W_q,W_k,W_v}[1024,1024] -> 16-head causal attention
with RoPE -> @ W_o[1024,1024].

Sharding (batch x heads): core c handles batch b=c//4 and head group g=c%4
(4 heads = 256 of the 1024 qkv dims). Each core computes its heads' QKV
projections, RoPE, causal attention, and a partial out-projection
(ctx_g @ W_o[256g:256g+256, :]). The host sums the 4 partials per batch
(unshard of a partial-sum sharding) and transposes back.

On-device layout is fully transposed ([feature, seq]) so no transposes are
needed anywhere: scores are computed as scoresT[k,q] = K^T.T @ Q^T, the
softmax denominator falls out of the AV matmul via a ones-column appended to
V, and the out-projection consumes ctxT directly.

Score/attn tiles are laid out [128 keys, (head 2, q 512)] so the
causally-dead low-q range of diagonal key-blocks is never computed: the
score matmuls, the exp, and the AV matmuls all start at query offset
128*j for diagonal block j (exact - the skipped entries are masked to
zero in the reference too). Only the 128-wide q sub-range at the
diagonal needs masking, with a single persistent [128, 2*128] k<=q mask
(matmul operands stay contiguous 2D; only exp/mask use strided APs).

Softmax normalization: the denominator row (PSUM partition 64, from the
ones-column) is reciprocal'd in place by the DVE, broadcast across
partitions by a K=1 ones-matmul on the PE (one 512-row pass), and applied
by one DVE multiply - no DRAM bounce, no repartition DMAs.

The mask multiplies (SBUF-only) run on the otherwise-idle Pool engine
(GPSIMD cannot read PSUM, so all PSUM->SBUF staging stays on DVE/ACT);
a quarter of the out-projection casts go to ACT. The head-pair-0
normalization is emitted INSIDE attention(1) (after the kb==1 scores):
the ctx PSUM buffers are recycled by the next attention call, so their
consumers must precede that call's AV matmuls in every engine queue.

The whole kernel is one software pipeline over seq blocks sb; throwaway
matmuls warm the PE's HAM clock gate during the input load and the tail.
"""

import numpy as np
import ml_dtypes

B = 2
S = 2048
D = 1024
H = 16
HD = 64
N_CORES = 8
H_PER_CORE = 4
DQ = H_PER_CORE * HD  # 256 qkv dims per core
N_DC = D // 128  # 8 contraction chunks
N_SB = S // 512  # 4 seq blocks of 512
N_KB = S // 128  # 16 key blocks of 128
THETA = 10000.0

_CACHED = None


def _build_kernel():
    import concourse.bass as bass
    import concourse.mybir as mybir
    import concourse.tile as tile
    from concourse import bacc

    f32 = mybir.dt.float32
    bf16 = mybir.dt.bfloat16

    nc = bacc.Bacc(None, target_bir_lowering=False, num_devices=N_CORES)

    xT = nc.dram_tensor("xT", [D, S], bf16, kind="ExternalInput")
    wq = nc.dram_tensor("wq", [D, DQ], bf16, kind="ExternalInput")
    wk = nc.dram_tensor("wk", [D, DQ], bf16, kind="ExternalInput")
    wv = nc.dram_tensor("wv", [D, DQ], bf16, kind="ExternalInput")
    wo = nc.dram_tensor("wo", [DQ, D], bf16, kind="ExternalInput")
    cosT = nc.dram_tensor("cosT", [128, S], f32, kind="ExternalInput")
    sinT = nc.dram_tensor("sinT", [128, S], f32, kind="ExternalInput")
    # mask[k, 128*h + q] = 1.0 if k <= q else 0 (h=0,1 same)
    masks = nc.dram_tensor("masks", [128, 256], bf16, kind="ExternalInput")
    yT = nc.dram_tensor("yT", [D, S], bf16, kind="ExternalOutput")

    with tile.TileContext(nc) as tc:
        with (
            tc.tile_pool(name="persist", bufs=1) as persist,
            tc.tile_pool(name="attn", bufs=8) as attn_pool,
            tc.tile_pool(name="rope", bufs=4) as rope_pool,
            tc.tile_pool(name="small", bufs=4) as small_pool,
            tc.tile_pool(name="yout", bufs=3) as yout_pool,
            tc.tile_pool(name="psA", bufs=2, space="PSUM") as psA,  # scores 2-bank
            tc.tile_pool(name="psB", bufs=2, space="PSUM") as psB,  # ctx accum
            tc.tile_pool(name="psC", bufs=2, space="PSUM") as psC,  # proj/y
        ):
            # ---------------- input DMA ----------------
            # spread across both HWDGE rings (sync + scalar) and the
            # gpsimd SWDGE so the x chunks land as early as possible;
            # x/wq first (first projection), wv/wo/mask deferred.
            wq_sb = persist.tile([128, N_DC, DQ], bf16, tag="wq")
            nc.sync.dma_start(
                out=wq_sb[:], in_=wq.rearrange("(c p) n -> p c n", p=128)
            )
            wk_sb = persist.tile([128, N_DC, DQ], bf16, tag="wk")
            nc.scalar.dma_start(
                out=wk_sb[:], in_=wk.rearrange("(c p) n -> p c n", p=128)
            )
            xt_sb = [
                persist.tile([128, S], bf16, tag=f"xt{dc}", name=f"xt{dc}")
                for dc in range(N_DC)
            ]
            engs = [nc.sync, nc.scalar, nc.gpsimd]
            for dc in range(N_DC):
                engs[dc % 3].dma_start(
                    out=xt_sb[dc][:], in_=xT[128 * dc : 128 * (dc + 1), :]
                )
            cos_sb = persist.tile([128, S], f32, tag="cos")
            sin_sb = persist.tile([128, S], f32, tag="sin")
            nc.gpsimd.dma_start(out=cos_sb[:, 0:512], in_=cosT[:, 0:512])
            nc.sync.dma_start(out=sin_sb[:, 0:512], in_=sinT[:, 0:512])
            wv_sb = persist.tile([128, N_DC, DQ], bf16, tag="wv")
            nc.scalar.dma_start(
                out=wv_sb[:], in_=wv.rearrange("(c p) n -> p c n", p=128)
            )
            nc.sync.dma_start(out=cos_sb[:, 512:S], in_=cosT[:, 512:S])
            nc.scalar.dma_start(out=sin_sb[:, 512:S], in_=sinT[:, 512:S])
            mask_sb = persist.tile([128, 2, 128], bf16, tag="mask")
            nc.gpsimd.dma_start(
                out=mask_sb[:], in_=masks.rearrange("p (h q) -> p h q", h=2)
            )
            wo_sb = persist.tile([128, 2, D], bf16, tag="wo")
            nc.sync.dma_start(
                out=wo_sb[:], in_=wo.rearrange("(c p) n -> p c n", p=128)
            )
            ones_sb = persist.tile([1, 128], bf16, tag="ones")
            nc.vector.memset(ones_sb[:], 1.0)

            # PE warm-up: the HAM clock gate needs ~3.4us of sustained
            # activity to lift the PE to 2.4GHz; run throwaway matmuls on the
            # first-arrived weight tile while x is still streaming in
            warm0 = psA.tile([128, 2, 512], f32, tag="score", name="warm0")
            for wi in range(40):
                nc.tensor.matmul(
                    warm0[:, 0, 0:256],
                    wq_sb[:, 0, 0:128],
                    wq_sb[:, wi % 4, :],
                    start=True,
                    stop=True,
                )
            # pre-load the Exp activation table while ACT is idle so the
            # first real exp doesn't pay the ~1.3us table swap
            dummy_sb = small_pool.tile([1, 8], bf16, tag="dummy", name="dummy")
            nc.scalar.activation(
                dummy_sb[:], ones_sb[0:1, 0:8], mybir.ActivationFunctionType.Exp
            )

            # persistent intermediates
            qT_sb = persist.tile([128, 2, S], bf16, tag="qT")  # [64h..., cc, s]
            kT_sb = persist.tile([128, 2, S], bf16, tag="kT")
            v_sb = persist.tile([128, N_KB, H_PER_CORE, HD + 1], bf16, tag="v")
            nc.vector.memset(v_sb[:, :, :, HD : HD + 1], 1.0)
            ctxT_sb = persist.tile([128, 2, S], bf16, tag="ctxT")  # unnormalized

            # ---------------- helpers ----------------
            def rope_pair(q_ps, k_ps, cc, sb):
                """qT/kT = src*cos + rotate_half(src)*sin, fp32 in, bf16 out.

                q and k are processed together so the rotate-half partition
                shift is 4 SBUF->SBUF DMAs of [32, 1024] instead of 8 of
                [32, 512] (a [32,*] DVE op costs as much as a [128,*] one,
                so quarter-sized DVE ops waste 3/4 of the lanes; DMA engines
                are otherwise idle and the issue count is what matters).
                """
                ss = slice(512 * sb, 512 * (sb + 1))
                t1 = rope_pool.tile([128, 2, 512], bf16, tag="ropeA", name="t1")
                nc.vector.tensor_mul(t1[:, 0, :], q_ps[:], cos_sb[:, ss])
                nc.vector.tensor_mul(t1[:, 1, :], k_ps[:], cos_sb[:, ss])
                # sin table is pre-shifted on the host (sinx[p] =
                # sin_signed[partner(p)]) so this product is computed at the
                # SOURCE rows and only then moved to the partner rows by DMA
                t2p = rope_pool.tile([128, 2, 512], bf16, tag="ropeQ", name="t2p")
                nc.vector.tensor_mul(t2p[:, 0, :], q_ps[:], sin_sb[:, ss])
                nc.vector.tensor_mul(t2p[:, 1, :], k_ps[:], sin_sb[:, ss])
                rot = rope_pool.tile([128, 2, 512], bf16, tag="ropeB", name="rot")
                for quarter in range(4):
                    o = 32 * quarter
                    src_o = o + 32 if quarter % 2 == 0 else o - 32
                    nc.gpsimd.dma_start(
                        out=rot[o : o + 32], in_=t2p[src_o : src_o + 32]
                    )
                nc.vector.tensor_add(qT_sb[:, cc, ss], t1[:, 0, :], rot[:, 0, :])
                nc.vector.tensor_add(kT_sb[:, cc, ss], t1[:, 1, :], rot[:, 1, :])

            def proj_qk(cc, sb):
                ss = slice(512 * sb, 512 * (sb + 1))
                q_ps = psC.tile([128, 512], f32, tag="proj", name="q_ps")
                for dc in range(N_DC):
                    nc.tensor.matmul(
                        q_ps[:],
                        wq_sb[:, dc, 128 * cc : 128 * (cc + 1)],
                        xt_sb[dc][:, ss],
                        start=(dc == 0),
                        stop=(dc == N_DC - 1),
                    )
                k_ps = psC.tile([128, 512], f32, tag="proj", name="k_ps")
                for dc in range(N_DC):
                    nc.tensor.matmul(
                        k_ps[:],
                        wk_sb[:, dc, 128 * cc : 128 * (cc + 1)],
                        xt_sb[dc][:, ss],
                        start=(dc == 0),
                        stop=(dc == N_DC - 1),
                    )
                rope_pair(q_ps, k_ps, cc, sb)

            def proj_v(sc):
                v_ps = psC.tile([128, DQ], f32, tag="proj", name="v_ps")
                for dc in range(N_DC):
                    nc.tensor.matmul(
                        v_ps[:],
                        xt_sb[dc][:, 128 * sc : 128 * (sc + 1)],
                        wv_sb[:, dc, :],
                        start=(dc == 0),
                        stop=(dc == N_DC - 1),
                    )
                nc.vector.tensor_copy(
                    v_sb[:, sc, :, 0:HD],
                    v_ps[:].rearrange("p (h d) -> p h d", h=H_PER_CORE),
                )

            def attention(cc, qb, early=None, filler=None):
                """Causal attention for head pair cc, query block qb.

                Score/attn tiles are [128 k, (qj 4, h 2, qw 128)]; diagonal
                key-block j only computes query sub-blocks qj >= j (the rest
                is causally dead), and only sub-block qj==j is masked.
                Per k-block: two score matmuls (head h in PE row-group h),
                one exp over both heads, then (one k-block delayed) the two
                AV matmuls accumulating ctx+denominator via the ones column.

                `early` is emitted inside the kb==1 iteration, before the
                first AV matmuls: the previous head pair's normalization
                goes here so its ctx PSUM buffers are released before this
                call's AV needs them, with the kb 0-1 scores covering the
                reciprocal latency. `filler` is a list of callables emitting
                independent PE work at the end to cover pipeline bubbles.
                """
                nkb = 4 * qb + 4
                filler = list(filler or [])
                ctx_ps = [
                    psB.tile([HD + 1, 512], f32, tag="ctx", name=f"ctx{h}")
                    for h in range(2)
                ]
                pending = None  # (kb, j0, attnT tile) whose AV hasn't run
                for kb in range(nkb):
                    j = kb - 4 * qb  # >=0 on diagonal blocks
                    j0 = max(j, 0)
                    s_ps = psA.tile([128, 2, 512], f32, tag="score", name="s_ps")
                    for h in range(2):
                        hp = slice(64 * h, 64 * (h + 1))
                        nc.tensor.matmul(
                            s_ps[:, h, 128 * j0 : 512],
                            kT_sb[hp, cc, 128 * kb : 128 * (kb + 1)],
                            qT_sb[hp, cc, 512 * qb + 128 * j0 : 512 * (qb + 1)],
                            start=True,
                            stop=True,
                        )
                    if kb == 1 and early is not None:
                        early()
                    a_t = attn_pool.tile(
                        [128, 2, 512], bf16, tag="attnT", name="a_t"
                    )
                    nc.scalar.activation(
                        a_t[:, :, 128 * j0 : 512],
                        s_ps[:, :, 128 * j0 : 512],
                        mybir.ActivationFunctionType.Exp,
                        scale=float(1.0 / np.sqrt(HD)),
                    )
                    if j >= 0:
                        # Pool engine: SBUF-only op, keeps the DVE free
                        nc.gpsimd.tensor_mul(
                            a_t[:, :, 128 * j : 128 * (j + 1)],
                            a_t[:, :, 128 * j : 128 * (j + 1)],
                            mask_sb[:],
                        )
                    if pending is not None:
                        pkb, pj0, p_t = pending
                        for h in range(2):
                            nc.tensor.matmul(
                                ctx_ps[h][:, 128 * pj0 : 512],
                                v_sb[:, pkb, 2 * cc + h, :],
                                p_t[:, h, 128 * pj0 : 512],
                                start=(pkb == 0),
                                stop=False,
                            )
                    pending = (kb, j0, a_t)
                pkb, pj0, p_t = pending
                for h in range(2):
                    nc.tensor.matmul(
                        ctx_ps[h][:, 128 * pj0 : 512],
                        v_sb[:, pkb, 2 * cc + h, :],
                        p_t[:, h, 128 * pj0 : 512],
                        start=(pkb == 0),
                        stop=True,
                    )
                for f in filler:
                    f()
                return ctx_ps

            def recip(ctx_ps):
                """Reciprocal of the denominator rows (PSUM partition 64)."""
                rec = small_pool.tile([1, 2, 512], bf16, tag="rec", name="rec")
                with nc.allow_low_precision(
                    reason="bf16 softmax denom matches bf16 attn weights"
                ):
                    for h in range(2):
                        nc.vector.reciprocal(
                            rec[0:1, h, :], ctx_ps[h][HD : HD + 1, :]
                        )
                return rec

            def finish_norm(ctx_ps, rec, cc, qb):
                """Broadcast recip across partitions via K=1 ones-matmuls,
                stage ctx PSUM->SBUF on the Pool engine, scale ctxT."""
                qs = slice(512 * qb, 512 * (qb + 1))
                bc_ps = psC.tile([128, 512], f32, tag="proj", name="bc_ps")
                for h in range(2):
                    nc.tensor.matmul(
                        bc_ps[64 * h : 64 * (h + 1), :],
                        ones_sb[0:1, 0:64],
                        rec[0:1, h, :],
                        start=True,
                        stop=True,
                    )
                for h in range(2):
                    nc.vector.tensor_copy(
                        ctxT_sb[64 * h : 64 * (h + 1), cc, qs],
                        ctx_ps[h][0:HD, :],
                    )
                nc.vector.tensor_mul(
                    ctxT_sb[:, cc, qs], ctxT_sb[:, cc, qs], bc_ps[:]
                )

            def out_proj(qb, ocs):
                qs = slice(512 * qb, 512 * (qb + 1))
                for oc in ocs:
                    y_ps = psC.tile([128, 512], f32, tag="proj", name="y_ps")
                    for cc in range(2):
                        nc.tensor.matmul(
                            y_ps[:],
                            wo_sb[:, cc, 128 * oc : 128 * (oc + 1)],
                            ctxT_sb[:, cc, qs],
                            start=(cc == 0),
                            stop=(cc == 1),
                        )
                    y_sb = yout_pool.tile([128, 512], bf16, tag="y", name="y_sb")
                    if oc % 4 == 3:
                        nc.scalar.copy(y_sb[:], y_ps[:])
                    else:
                        nc.vector.tensor_copy(y_sb[:], y_ps[:])
                    nc.sync.dma_start(
                        out=yT[128 * oc : 128 * (oc + 1), qs], in_=y_sb[:]
                    )

            # ---------------- main pipeline ----------------
            proj_qk(0, 0)
            for sb in range(N_SB):
                proj_qk(1, sb)
                for sc in range(4 * sb, 4 * sb + 4):
                    proj_v(sc)
                ctx0 = attention(0, sb)
                rec0 = recip(ctx0)
                fill = (
                    [(lambda oc=oc: out_proj(2, [oc])) for oc in range(4, N_DC)]
                    if sb == N_SB - 1
                    else None
                )
                ctx1 = attention(
                    1,
                    sb,
                    early=lambda: finish_norm(ctx0, rec0, 0, sb),
                    filler=fill,
                )
                rec1 = recip(ctx1)
                if sb < N_SB - 1:
                    # emit the next block's first projection before this
                    # block's bc1/out-projection so the reciprocal chain is
                    # covered by PE work and the PE never idles
                    proj_qk(0, sb + 1)
                finish_norm(ctx1, rec1, 1, sb)
                if sb < 2:
                    out_proj(sb, range(N_DC))
                elif sb == 2:
                    # hold back half of qb=2's out-projection; it is emitted
                    # as filler inside attention(1, 3) to cover the final
                    # normalization chain
                    out_proj(2, range(0, 4))
                else:
                    out_proj(3, range(N_DC))

    nc.compile()
    return nc


def _rope_tables():
    inv_freq = (
        1.0 / (THETA ** (np.arange(0, HD, 2, dtype=np.float32) / HD))
    ).astype(np.float32)
    pos = np.arange(S, dtype=np.float32)
    ang = pos[:, None] * inv_freq[None, :]  # [S, 32]
    cos_half = np.cos(ang).astype(np.float32).T  # [32, S]
    sin_half = np.sin(ang).astype(np.float32).T
    # per-head 64 rows: cos rows duplicated. The sin table is PRE-SHIFTED:
    # row p holds sin_signed[partner(p)] (partner = rotate-half swap), so the
    # kernel multiplies at the source rows and a plain partition-shift DMA
    # finishes rotate-half: sinx per head = (+sin | -sin).
    cos64 = np.concatenate([cos_half, cos_half], axis=0)
    sinx64 = np.concatenate([sin_half, -sin_half], axis=0)
    cosT = np.concatenate([cos64, cos64], axis=0)  # [128, S] two heads
    sinT = np.concatenate([sinx64, sinx64], axis=0)
    return np.ascontiguousarray(cosT), np.ascontiguousarray(sinT)


def _masks():
    k = np.arange(128)[:, None]
    q = np.arange(128)[None, :]
    blk = (k <= q).astype(ml_dtypes.bfloat16)
    return np.ascontiguousarray(np.concatenate([blk, blk], axis=1))


def kernel(x, W_q, W_k, W_v, W_o):
    global _CACHED
    from concourse.bass_utils import run_bass_kernel_spmd

    if _CACHED is None:
        _CACHED = _build_kernel()
    nc = _CACHED

    bf = ml_dtypes.bfloat16
    cosT, sinT = _rope_tables()
    masks = _masks()
    x = np.asarray(x)
    W_q, W_k, W_v, W_o = (np.asarray(w) for w in (W_q, W_k, W_v, W_o))
    xT = [np.ascontiguousarray(x[b].T).astype(bf) for b in range(B)]

    in_maps = []
    for c in range(N_CORES):
        b, g = divmod(c, 4)
        cols = slice(DQ * g, DQ * (g + 1))
        in_maps.append(
            {
                "xT": xT[b],
                "wq": np.ascontiguousarray(W_q[:, cols]).astype(bf),
                "wk": np.ascontiguousarray(W_k[:, cols]).astype(bf),
                "wv": np.ascontiguousarray(W_v[:, cols]).astype(bf),
                "wo": np.ascontiguousarray(W_o[cols, :]).astype(bf),
                "cosT": cosT,
                "sinT": sinT,
                "masks": masks,
            }
        )

    res = run_bass_kernel_spmd(nc, in_maps, core_ids=list(range(N_CORES)))
    kernel.last_results = res

    y = np.empty((B, S, D), dtype=np.float32)
    for b in range(B):
        acc = res.results[4 * b]["yT"].astype(np.float32)
        for g in range(1, 4):
            acc += res.results[4 * b + g]["yT"].astype(np.float32)
        y[b] = acc.T
    return y


# revision 14
# speedup vs baseline: 1.1414x; 1.1414x over previous
"""Multi-head attention (RoPE, causal) Trainium2 kernel, SPMD over 8 NeuronCores.

Problem: x[2,2048,1024] @ {W_q,W_k,W_v}[1024,1024] -> 16-head causal attention
with RoPE -> @ W_o[1024,1024].

Sharding (batch x heads): core c handles batch b=c//4 and head group g=c%4
(4 heads = 256 of the 1024 qkv dims). Each core computes its heads' QKV
projections, RoPE, causal attention, and a partial out-projection
(ctx_g @ W_o[256g:256g+256, :]). The host sums the 4 partials per batch
(unshard of a partial-sum sharding) and transposes back.

On-device layout is fully transposed ([feature, seq]) so no transposes are
needed anywhere: scores are computed as scoresT[k,q] = K^T.T @ Q^T, the
softmax denominator falls out of the AV matmul via a ones-column appended to
V, and the out-projection consumes ctxT directly.

Score/attn tiles are laid out [128 keys, (head 2, q 512)] so the
causally-dead low-q range of diagonal key-blocks is never computed: the
score matmuls, the exp, and the AV matmuls all start at query offset
128*j for diagonal block j (exact - the skipped entries are masked to
zero in the reference too). Only the 128-wide q sub-range at the
diagonal needs masking, with a single persistent [128, 2*128] k<=q mask
(matmul operands stay contiguous 2D; only exp/mask use strided APs).

Softmax normalization: the denominator rows (PSUM partition 64, from the
ones-column) are staged to one SBUF row, repartitioned to [8,128] by an
SBUF->SBUF DMA (reciprocal cost scales with free size only), reciprocal'd,
DMA'd back to one row, broadcast across partitions by K=1 ones-matmuls on
the PE (one 512-row pass per head), and applied by one DVE multiply - no
DRAM bounce. The chain is off the critical path: ctx PSUM buffers are
released by the PSUM->SBUF copies alone; only the out-projection waits on
the normalize multiply.

The mask multiplies (SBUF-only) run on the otherwise-idle Pool engine
(GPSIMD cannot read PSUM, so all PSUM->SBUF staging stays on DVE/ACT);
a quarter of the out-projection casts go to ACT.

The whole kernel is one software pipeline over seq blocks sb; throwaway
matmuls warm the PE's HAM clock gate during the input load and the tail.
"""

import numpy as np
import ml_dtypes

B = 2
S = 2048
D = 1024
H = 16
HD = 64
N_CORES = 8
H_PER_CORE = 4
DQ = H_PER_CORE * HD  # 256 qkv dims per core
N_DC = D // 128  # 8 contraction chunks
N_SB = S // 512  # 4 seq blocks of 512
N_KB = S // 128  # 16 key blocks of 128
THETA = 10000.0

_CACHED = None


def _build_kernel():
    import concourse.bass as bass
    import concourse.mybir as mybir
    import concourse.tile as tile
    from concourse import bacc

    f32 = mybir.dt.float32
    bf16 = mybir.dt.bfloat16

    nc = bacc.Bacc(None, target_bir_lowering=False, num_devices=N_CORES)

    xT = nc.dram_tensor("xT", [D, S], bf16, kind="ExternalInput")
    wq = nc.dram_tensor("wq", [D, DQ], bf16, kind="ExternalInput")
    wk = nc.dram_tensor("wk", [D, DQ], bf16, kind="ExternalInput")
    wv = nc.dram_tensor("wv", [D, DQ], bf16, kind="ExternalInput")
    wo = nc.dram_tensor("wo", [DQ, D], bf16, kind="ExternalInput")
    cosT = nc.dram_tensor("cosT", [128, S], f32, kind="ExternalInput")
    sinT = nc.dram_tensor("sinT", [128, S], f32, kind="ExternalInput")
    # mask[k, 128*h + q] = 1.0 if k <= q else 0 (h=0,1 same)
    masks = nc.dram_tensor("masks", [128, 256], bf16, kind="ExternalInput")
    yT = nc.dram_tensor("yT", [D, S], bf16, kind="ExternalOutput")

    with tile.TileContext(nc) as tc:
        with (
            tc.tile_pool(name="persist", bufs=1) as persist,
            tc.tile_pool(name="attn", bufs=8) as attn_pool,
            tc.tile_pool(name="rope", bufs=4) as rope_pool,
            tc.tile_pool(name="small", bufs=4) as small_pool,
            tc.tile_pool(name="yout", bufs=3) as yout_pool,
            tc.tile_pool(name="psA", bufs=2, space="PSUM") as psA,  # scores 2-bank
            tc.tile_pool(name="psB", bufs=2, space="PSUM") as psB,  # ctx accum
            tc.tile_pool(name="psC", bufs=2, space="PSUM") as psC,  # proj/y
        ):
            # ---------------- input DMA ----------------
            # spread across both HWDGE rings (sync + scalar) and the
            # gpsimd SWDGE so the x chunks land as early as possible;
            # x/wq first (first projection), wv/wo/mask deferred.
            wq_sb = persist.tile([128, N_DC, DQ], bf16, tag="wq")
            nc.sync.dma_start(
                out=wq_sb[:], in_=wq.rearrange("(c p) n -> p c n", p=128)
            )
            wk_sb = persist.tile([128, N_DC, DQ], bf16, tag="wk")
            nc.scalar.dma_start(
                out=wk_sb[:], in_=wk.rearrange("(c p) n -> p c n", p=128)
            )
            xt_sb = [
                persist.tile([128, S], bf16, tag=f"xt{dc}", name=f"xt{dc}")
                for dc in range(N_DC)
            ]
            engs = [nc.sync, nc.scalar, nc.gpsimd]
            for dc in range(N_DC):
                engs[dc % 3].dma_start(
                    out=xt_sb[dc][:], in_=xT[128 * dc : 128 * (dc + 1), :]
                )
            cos_sb = persist.tile([128, S], f32, tag="cos")
            sin_sb = persist.tile([128, S], f32, tag="sin")
            nc.gpsimd.dma_start(out=cos_sb[:, 0:512], in_=cosT[:, 0:512])
            nc.sync.dma_start(out=sin_sb[:, 0:512], in_=sinT[:, 0:512])
            wv_sb = persist.tile([128, N_DC, DQ], bf16, tag="wv")
            nc.scalar.dma_start(
                out=wv_sb[:], in_=wv.rearrange("(c p) n -> p c n", p=128)
            )
            nc.sync.dma_start(out=cos_sb[:, 512:S], in_=cosT[:, 512:S])
            nc.scalar.dma_start(out=sin_sb[:, 512:S], in_=sinT[:, 512:S])
            mask_sb = persist.tile([128, 2, 128], bf16, tag="mask")
            nc.gpsimd.dma_start(
                out=mask_sb[:], in_=masks.rearrange("p (h q) -> p h q", h=2)
            )
            wo_sb = persist.tile([128, 2, D], bf16, tag="wo")
            nc.sync.dma_start(
                out=wo_sb[:], in_=wo.rearrange("(c p) n -> p c n", p=128)
            )
            ones_sb = persist.tile([1, 128], bf16, tag="ones")
            nc.vector.memset(ones_sb[:], 1.0)

            # PE warm-up: the HAM clock gate needs ~3.4us of sustained
            # activity to lift the PE to 2.4GHz; run throwaway matmuls on the
            # first-arrived weight tile while x is still streaming in
            warm0 = psA.tile([128, 2, 512], f32, tag="score", name="warm0")
            for wi in range(40):
                nc.tensor.matmul(
                    warm0[:, 0, 0:256],
                    wq_sb[:, 0, 0:128],
                    wq_sb[:, wi % 4, :],
                    start=True,
                    stop=True,
                )
            # pre-load the Exp activation table while ACT is idle so the
            # first real exp doesn't pay the ~1.3us table swap
            dummy_sb = small_pool.tile([1, 8], bf16, tag="dummy", name="dummy")
            nc.scalar.activation(
                dummy_sb[:], ones_sb[0:1, 0:8], mybir.ActivationFunctionType.Exp
            )

            # persistent intermediates
            qT_sb = persist.tile([128, 2, S], bf16, tag="qT")  # [64h..., cc, s]
            kT_sb = persist.tile([128, 2, S], bf16, tag="kT")
            v_sb = persist.tile([128, N_KB, H_PER_CORE, HD + 1], bf16, tag="v")
            nc.vector.memset(v_sb[:, :, :, HD : HD + 1], 1.0)
            ctxT_sb = persist.tile([128, 2, S], bf16, tag="ctxT")  # unnormalized

            # ---------------- helpers ----------------
            def rope_pair(q_ps, k_ps, cc, sb):
                """qT/kT = src*cos + rotate_half(src)*sin, fp32 in, bf16 out.

                q and k are processed together so the rotate-half partition
                shift is 4 SBUF->SBUF DMAs of [32, 1024] instead of 8 of
                [32, 512] (a [32,*] DVE op costs as much as a [128,*] one,
                so quarter-sized DVE ops waste 3/4 of the lanes; DMA engines
                are otherwise idle and the issue count is what matters).
                """
                ss = slice(512 * sb, 512 * (sb + 1))
                t1 = rope_pool.tile([128, 2, 512], bf16, tag="ropeA", name="t1")
                nc.vector.tensor_mul(t1[:, 0, :], q_ps[:], cos_sb[:, ss])
                nc.vector.tensor_mul(t1[:, 1, :], k_ps[:], cos_sb[:, ss])
                # sin table is pre-shifted on the host (sinx[p] =
                # sin_signed[partner(p)]) so this product is computed at the
                # SOURCE rows and only then moved to the partner rows by DMA
                t2p = rope_pool.tile([128, 2, 512], bf16, tag="ropeQ", name="t2p")
                nc.vector.tensor_mul(t2p[:, 0, :], q_ps[:], sin_sb[:, ss])
                nc.vector.tensor_mul(t2p[:, 1, :], k_ps[:], sin_sb[:, ss])
                rot = rope_pool.tile([128, 2, 512], bf16, tag="ropeB", name="rot")
                for quarter in range(4):
                    o = 32 * quarter
                    src_o = o + 32 if quarter % 2 == 0 else o - 32
                    nc.gpsimd.dma_start(
                        out=rot[o : o + 32], in_=t2p[src_o : src_o + 32]
                    )
                nc.vector.tensor_add(qT_sb[:, cc, ss], t1[:, 0, :], rot[:, 0, :])
                nc.vector.tensor_add(kT_sb[:, cc, ss], t1[:, 1, :], rot[:, 1, :])

            def proj_qk(cc, sb):
                ss = slice(512 * sb, 512 * (sb + 1))
                q_ps = psC.tile([128, 512], f32, tag="proj", name="q_ps")
                for dc in range(N_DC):
                    nc.tensor.matmul(
                        q_ps[:],
                        wq_sb[:, dc, 128 * cc : 128 * (cc + 1)],
                        xt_sb[dc][:, ss],
                        start=(dc == 0),
                        stop=(dc == N_DC - 1),
                    )
                k_ps = psC.tile([128, 512], f32, tag="proj", name="k_ps")
                for dc in range(N_DC):
                    nc.tensor.matmul(
                        k_ps[:],
                        wk_sb[:, dc, 128 * cc : 128 * (cc + 1)],
                        xt_sb[dc][:, ss],
                        start=(dc == 0),
                        stop=(dc == N_DC - 1),
                    )
                rope_pair(q_ps, k_ps, cc, sb)

            def proj_v(sc):
                v_ps = psC.tile([128, DQ], f32, tag="proj", name="v_ps")
                for dc in range(N_DC):
                    nc.tensor.matmul(
                        v_ps[:],
                        xt_sb[dc][:, 128 * sc : 128 * (sc + 1)],
                        wv_sb[:, dc, :],
                        start=(dc == 0),
                        stop=(dc == N_DC - 1),
                    )
                nc.vector.tensor_copy(
                    v_sb[:, sc, :, 0:HD],
                    v_ps[:].rearrange("p (h d) -> p h d", h=H_PER_CORE),
                )

            def attention(cc, qb, filler=None):
                """Causal attention for head pair cc, query block qb.

                Score/attn tiles are [128 k, (qj 4, h 2, qw 128)]; diagonal
                key-block j only computes query sub-blocks qj >= j (the rest
                is causally dead), and only sub-block qj==j is masked.
                Per k-block: two score matmuls (head h in PE row-group h),
                one exp over both heads, then (one k-block delayed) the two
                AV matmuls accumulating ctx+denominator via the ones column.

                `filler` is a list of callables emitting independent PE
                work at the end to cover pipeline bubbles.
                """
                nkb = 4 * qb + 4
                filler = list(filler or [])
                ctx_ps = [
                    psB.tile([HD + 1, 512], f32, tag="ctx", name=f"ctx{h}")
                    for h in range(2)
                ]
                pending = None  # (kb, j0, attnT tile) whose AV hasn't run
                for kb in range(nkb):
                    j = kb - 4 * qb  # >=0 on diagonal blocks
                    j0 = max(j, 0)
                    s_ps = psA.tile([128, 2, 512], f32, tag="score", name="s_ps")
                    for h in range(2):
                        hp = slice(64 * h, 64 * (h + 1))
                        nc.tensor.matmul(
                            s_ps[:, h, 128 * j0 : 512],
                            kT_sb[hp, cc, 128 * kb : 128 * (kb + 1)],
                            qT_sb[hp, cc, 512 * qb + 128 * j0 : 512 * (qb + 1)],
                            start=True,
                            stop=True,
                        )
                    a_t = attn_pool.tile(
                        [128, 2, 512], bf16, tag="attnT", name="a_t"
                    )
                    nc.scalar.activation(
                        a_t[:, :, 128 * j0 : 512],
                        s_ps[:, :, 128 * j0 : 512],
                        mybir.ActivationFunctionType.Exp,
                        scale=float(1.0 / np.sqrt(HD)),
                    )
                    if j >= 0:
                        # Pool engine: SBUF-only op, keeps the DVE free
                        nc.gpsimd.tensor_mul(
                            a_t[:, :, 128 * j : 128 * (j + 1)],
                            a_t[:, :, 128 * j : 128 * (j + 1)],
                            mask_sb[:],
                        )
                    if pending is not None:
                        pkb, pj0, p_t = pending
                        for h in range(2):
                            nc.tensor.matmul(
                                ctx_ps[h][:, 128 * pj0 : 512],
                                v_sb[:, pkb, 2 * cc + h, :],
                                p_t[:, h, 128 * pj0 : 512],
                                start=(pkb == 0),
                                stop=False,
                            )
                    pending = (kb, j0, a_t)
                pkb, pj0, p_t = pending
                for h in range(2):
                    nc.tensor.matmul(
                        ctx_ps[h][:, 128 * pj0 : 512],
                        v_sb[:, pkb, 2 * cc + h, :],
                        p_t[:, h, 128 * pj0 : 512],
                        start=(pkb == 0),
                        stop=True,
                    )
                for f in filler:
                    f()
                return ctx_ps

            def drain(ctx_ps, cc, qb):
                """Release the ctx PSUM tiles and start the reciprocal chain.

                Emitted right after attention(cc): the PSUM->SBUF copies and
                denominator stages are the only readers of ctx_ps, so the
                next attention's AV can recycle the banks immediately. The
                repartition DMA -> reciprocal -> row DMA runs off to the
                side; its result is consumed by norm_apply later.
                """
                qs = slice(512 * qb, 512 * (qb + 1))
                stage = small_pool.tile([1, 2, 512], f32, tag="stage", name="stage")
                nc.vector.tensor_copy(stage[0:1, 0, :], ctx_ps[0][HD : HD + 1, :])
                nc.scalar.copy(stage[0:1, 1, :], ctx_ps[1][HD : HD + 1, :])
                for h in range(2):
                    nc.vector.tensor_copy(
                        ctxT_sb[64 * h : 64 * (h + 1), cc, qs],
                        ctx_ps[h][0:HD, :],
                    )
                den_q = small_pool.tile([8, 128], f32, tag="den_q", name="den_q")
                nc.sync.dma_start(out=den_q[:], in_=stage[0:1, :, :])
                rec_q = small_pool.tile([8, 128], bf16, tag="rec_q", name="rec_q")
                with nc.allow_low_precision(
                    reason="bf16 softmax denom matches bf16 attn weights"
                ):
                    nc.vector.reciprocal(rec_q[:], den_q[:])
                rec_row = small_pool.tile([1, 2, 512], bf16, tag="rec", name="rec")
                nc.sync.dma_start(out=rec_row[0:1, :, :], in_=rec_q[:])
                return rec_row

            def norm_apply(rec_row, cc, qb):
                """Broadcast recip across partitions via K=1 ones-matmuls,
                then scale ctxT. Only the out-projection depends on this."""
                qs = slice(512 * qb, 512 * (qb + 1))
                bc_ps = psC.tile([128, 512], f32, tag="proj", name="bc_ps")
                for h in range(2):
                    nc.tensor.matmul(
                        bc_ps[64 * h : 64 * (h + 1), :],
                        ones_sb[0:1, 0:64],
                        rec_row[0:1, h, :],
                        start=True,
                        stop=True,
                    )
                nc.vector.tensor_mul(
                    ctxT_sb[:, cc, qs], ctxT_sb[:, cc, qs], bc_ps[:]
                )

            def out_proj(qb, ocs):
                qs = slice(512 * qb, 512 * (qb + 1))
                for oc in ocs:
                    y_ps = psC.tile([128, 512], f32, tag="proj", name="y_ps")
                    for cc in range(2):
                        nc.tensor.matmul(
                            y_ps[:],
                            wo_sb[:, cc, 128 * oc : 128 * (oc + 1)],
                            ctxT_sb[:, cc, qs],
                            start=(cc == 0),
                            stop=(cc == 1),
                        )
                    y_sb = yout_pool.tile([128, 512], bf16, tag="y", name="y_sb")
                    if oc % 4 == 3:
                        nc.scalar.copy(y_sb[:], y_ps[:])
                    else:
                        nc.vector.tensor_copy(y_sb[:], y_ps[:])
                    nc.sync.dma_start(
                        out=yT[128 * oc : 128 * (oc + 1), qs], in_=y_sb[:]
                    )

            # ---------------- main pipeline ----------------
            proj_qk(0, 0)
            for sb in range(N_SB):
                proj_qk(1, sb)
                for sc in range(4 * sb, 4 * sb + 4):
                    proj_v(sc)
                ctx0 = attention(0, sb)
                rec0 = drain(ctx0, 0, sb)
                fill = (
                    [(lambda oc=oc: out_proj(2, [oc])) for oc in range(4, N_DC)]
                    if sb == N_SB - 1
                    else None
                )
                ctx1 = attention(1, sb, filler=fill)
                # bc0 lands on the PE right after attention(1)'s matmuls; the
                # reciprocal chain completed during attention(1)
                norm_apply(rec0, 0, sb)
                rec1 = drain(ctx1, 1, sb)
                if sb < N_SB - 1:
                    # emit the next block's first projection before this
                    # block's bc1/out-projection so the reciprocal chain is
                    # covered by PE work and the PE never idles
                    proj_qk(0, sb + 1)
                norm_apply(rec1, 1, sb)
                if sb < 2:
                    out_proj(sb, range(N_DC))
                elif sb == 2:
                    # hold back half of qb=2's out-projection; it is emitted
                    # as filler inside attention(1, 3) to cover the final
                    # normalization chain
                    out_proj(2, range(0, 4))
                else:
                    out_proj(3, range(N_DC))

    nc.compile()
    return nc


def _rope_tables():
    inv_freq = (
        1.0 / (THETA ** (np.arange(0, HD, 2, dtype=np.float32) / HD))
    ).astype(np.float32)
    pos = np.arange(S, dtype=np.float32)
    ang = pos[:, None] * inv_freq[None, :]  # [S, 32]
    cos_half = np.cos(ang).astype(np.float32).T  # [32, S]
    sin_half = np.sin(ang).astype(np.float32).T
    # per-head 64 rows: cos rows duplicated. The sin table is PRE-SHIFTED:
    # row p holds sin_signed[partner(p)] (partner = rotate-half swap), so the
    # kernel multiplies at the source rows and a plain partition-shift DMA
    # finishes rotate-half: sinx per head = (+sin | -sin).
    cos64 = np.concatenate([cos_half, cos_half], axis=0)
    sinx64 = np.concatenate([sin_half, -sin_half], axis=0)
    cosT = np.concatenate([cos64, cos64], axis=0)  # [128, S] two heads
    sinT = np.concatenate([sinx64, sinx64], axis=0)
    return np.ascontiguousarray(cosT), np.ascontiguousarray(sinT)


def _masks():
    k = np.arange(128)[:, None]
    q = np.arange(128)[None, :]
    blk = (k <= q).astype(ml_dtypes.bfloat16)
    return np.ascontiguousarray(np.concatenate([blk, blk], axis=1))


def kernel(x, W_q, W_k, W_v, W_o):
    global _CACHED
    from concourse.bass_utils import run_bass_kernel_spmd

    if _CACHED is None:
        _CACHED = _build_kernel()
    nc = _CACHED

    bf = ml_dtypes.bfloat16
    cosT, sinT = _rope_tables()
    masks = _masks()
    x = np.asarray(x)
    W_q, W_k, W_v, W_o = (np.asarray(w) for w in (W_q, W_k, W_v, W_o))
    xT = [np.ascontiguousarray(x[b].T).astype(bf) for b in range(B)]

    in_maps = []
    for c in range(N_CORES):
        b, g = divmod(c, 4)
        cols = slice(DQ * g, DQ * (g + 1))
        in_maps.append(
            {
                "xT": xT[b],
                "wq": np.ascontiguousarray(W_q[:, cols]).astype(bf),
                "wk": np.ascontiguousarray(W_k[:, cols]).astype(bf),
                "wv": np.ascontiguousarray(W_v[:, cols]).astype(bf),
                "wo": np.ascontiguousarray(W_o[cols, :]).astype(bf),
                "cosT": cosT,
                "sinT": sinT,
                "masks": masks,
            }
        )

    res = run_bass_kernel_spmd(nc, in_maps, core_ids=list(range(N_CORES)))
    kernel.last_results = res

    y = np.empty((B, S, D), dtype=np.float32)
    for b in range(B):
        acc = res.results[4 * b]["yT"].astype(np.float32)
        for g in range(1, 4):
            acc += res.results[4 * b + g]["yT"].astype(np.float32)
        y[b] = acc.T
    return y
